# revision 19
# baseline (speedup 1.0000x reference)
"""Distributed attention kernel for 8 trn2 NeuronCores.

Problem: B=2, N=2048, C=1024, H=16, D=64 attention with relative position
bias, qkv projection and output projection.

Sharding: head-parallel, 2 heads per core, both batches on every core.
Each core computes a partial output projection (its 2 heads' contribution
to all 1024 output channels); the host sums the 8 partials.

v2 structure (software-pipelined, batch-sequential):
  A(b0) -> B(b0) [A(b1) chunks interleaved] -> B(b1) [C(b0) chunks
  interleaved] -> C(b1)
  - stage B processes 512-wide query chunks (nj); per (nj, mi) one psum
    tile holds both heads' scores^T ([128 m, A|B 512 n each]); ACT exp
    evicts to bf16 es; DVE multiplies exp(bias) in; two PV matmuls
    accumulate into a [65, 1024] psum pair (ones column -> denominators).
  - scores matmuls are row-packed (head A K-rows 0:64, head B 64:128)
    and run concurrently on the PE.
  - reciprocal of the 4096 per-batch denominators is batched: DRAM
    round-trip reshapes [1, 4096] -> [128, 32] so the DVE divide uses
    all partitions (the [1, N] form costs ~6.5us per call).
  - exp(bias) tiles stream from HBM as 1MB chunks (4 mi per load).
"""

import sys

import numpy as np
import ml_dtypes

sys.path.insert(0, "/opt/trn_rl_repo")

B, N, C = 2, 2048, 1024
H, D = 16, 64
SCALE = D**-0.5
NCORES = 8
HPC = H // NCORES  # heads per core = 2

bf16 = ml_dtypes.bfloat16

_graph_cache = {}


def _fix_sync_waits(nc):
    """Walrus in this toolchain accepts at most ONE sync wait on compute
    instructions (two on DMAs).  Tile emits more.  Fix up the built graph:
      - drop waits on the instruction's own scheduled proc (in-order
        execution of a proc makes them always-satisfied),
      - hoist remaining excess waits onto standalone NoOps inserted just
        before the instruction on the same engine.
    """
    from concourse import mybir
    from concourse.tile_sem_assignment import PROC_NAME_TO_IDX

    idx_to_proc = {v: k for k, v in PROC_NAME_TO_IDX.items()}
    fixid = [0]
    for fn in nc.m.functions:
        for bb in fn.blocks:
            insts = list(bb.instructions)
            out = []
            changed = False
            for inst in insts:
                si = inst.sync_info
                if not si or not si.on_wait:
                    out.append(inst)
                    continue
                own = idx_to_proc.get(inst.bass_scheduled_proc, None)
                waits = list(si.on_wait)
                if own is not None:
                    kept = [
                        w
                        for w in waits
                        if w.ant_name.rsplit("_", 1)[0] != own
                    ]
                else:
                    kept = waits
                limit = 1
                hoist = []
                if len(kept) > limit:
                    hoist = kept[: len(kept) - limit]
                    kept = kept[len(kept) - limit :]
                if len(kept) != len(waits) or hoist:
                    changed = True
                    for w in hoist:
                        fixid[0] += 1
                        nop = mybir.InstNoOp(
                            name=f"W-fix-{fixid[0]}",
                            ins=[],
                            outs=[],
                            engine=inst.engine,
                            bass_nofuse=True,
                            text_hint="wait-split",
                            sync_info=mybir.SyncInfo(on_wait=[w], on_update=[]),
                        )
                        out.append(nop)
                    si.on_wait = kept
                out.append(inst)
            if changed:
                bb.instructions = out


def _build_graph():
    import concourse.bass as bass
    import concourse.tile as tile
    from concourse import mybir

    EXP = mybir.ActivationFunctionType.Exp
    fp32 = mybir.dt.float32
    bfl = mybir.dt.bfloat16

    nc = bass.Bass()

    xt_d = nc.declare_dram_parameter("xt", [B, 8, 128, N], bfl, isOutput=False)
    wq_d = nc.declare_dram_parameter("wq", [128, 8, 128], bfl, isOutput=False)
    wk_d = nc.declare_dram_parameter("wk", [128, 8, 128], bfl, isOutput=False)
    wv_d = nc.declare_dram_parameter("wv", [128, 8, 128], bfl, isOutput=False)
    bq_d = nc.declare_dram_parameter("bq", [128, 1], fp32, isOutput=False)
    bv_d = nc.declare_dram_parameter("bv", [128, 128], fp32, isOutput=False)
    # exp(bias^T): [nj 4][miP 4][m^ 128][mi4 4][h 2][n 512]
    eb_d = nc.declare_dram_parameter(
        "expb", [4, 4, 128, 4, 2, 512], bfl, isOutput=False
    )
    pw_d = nc.declare_dram_parameter("pw", [128, 1024], bfl, isOutput=False)
    out_d = nc.declare_dram_parameter("out", [B, 16, 128, C], bfl, isOutput=True)

    with tile.TileContext(nc) as tc:
        with (
            tc.tile_pool(name="weights", bufs=1) as wpool,
            tc.tile_pool(name="xt", bufs=1) as xpool,
            tc.tile_pool(name="qkvt", bufs=1) as qkpool,
            tc.tile_pool(name="eb", bufs=3) as ebpool,
            tc.tile_pool(name="es", bufs=8) as espool,
            tc.tile_pool(name="outsb", bufs=4) as opool,
            tc.tile_pool(name="ps", bufs=1, space="PSUM") as pspool,
            tc.tile_pool(name="dramsc", bufs=1, space="DRAM") as dpool,
        ):
            # ---- persistent weights ----
            wq = wpool.tile([128, 8, 128], bfl)
            wk = wpool.tile([128, 8, 128], bfl)
            wv = wpool.tile([128, 8, 128], bfl)
            pw = wpool.tile([128, 1024], bfl)
            bq = wpool.tile([128, 1], fp32)
            bv = wpool.tile([128, 128], fp32)
            nc.sync.dma_start(out=wq[:], in_=wq_d[:])
            nc.sync.dma_start(out=wk[:], in_=wk_d[:])
            nc.sync.dma_start(out=wv[:], in_=wv_d[:])
            nc.sync.dma_start(out=pw[:], in_=pw_d[:])
            nc.sync.dma_start(out=bq[:], in_=bq_d[:])
            nc.sync.dma_start(out=bv[:], in_=bv_d[:])

            qt = [qkpool.tile([128, N], bfl, tag=f"qt{b}", name=f"qt{b}") for b in range(B)]
            kt = [qkpool.tile([128, N], bfl, tag=f"kt{b}", name=f"kt{b}") for b in range(B)]
            aot = [qkpool.tile([128, N], bfl, tag=f"aot{b}", name=f"aot{b}") for b in range(B)]
            rec2 = [qkpool.tile([128, N], fp32, tag=f"rec2{b}", name=f"rec2{b}") for b in range(B)]
            vaug = [
                qkpool.tile([128, 16, 65], bfl, tag=f"vaug{u}", name=f"vaug{u}") for u in range(B * HPC)
            ]
            for u in range(B * HPC):
                nc.vector.memset(vaug[u][:, :, 64:65], 1.0)
            # denominators, one tile per 1024-wide query half: [h, nj', n]
            den_h = [
                qkpool.tile([1, 2, 2, 512], fp32, tag=f"den{i}", name=f"den{i}")
                for i in range(2)
            ]
            den_t = qkpool.tile([128, 16], fp32, tag="den_t", name="den_t")
            rec_t = qkpool.tile([128, 16], fp32, tag="rec_t", name="rec_t")

            rec_d = dpool.tile([1, 4096], fp32, tag="rec_d", name="rec_d")

            xts = {}

            def load_x(b):
                for ci in range(8):
                    t = xpool.tile([128, N], bfl, tag=f"xt{ci}", name=f"x{b}_{ci}")
                    nc.sync.dma_start(out=t[:], in_=xt_d[b, ci])
                    xts[b, ci] = t

            def qk_chunk(b, ni, tag="aux", which="qk"):
                """q^T and/or k^T for one 512-wide token chunk."""
                nsl = slice(512 * ni, 512 * ni + 512)
                ps = pspool.tile([128, 1024], fp32, tag=tag, name=f"qk{b}_{ni}")
                for ci in range(8):
                    if "q" in which:
                        nc.tensor.matmul(
                            ps[:, 0:512], lhsT=wq[:, ci, :], rhs=xts[b, ci][:, nsl],
                            start=(ci == 0), stop=(ci == 7),
                        )
                    if "k" in which:
                        nc.tensor.matmul(
                            ps[:, 512:1024], lhsT=wk[:, ci, :], rhs=xts[b, ci][:, nsl],
                            start=(ci == 0), stop=(ci == 7),
                        )
                if "q" in which:
                    nc.vector.tensor_scalar_add(
                        out=qt[b][:, nsl], in0=ps[:, 0:512], scalar1=bq[:]
                    )
                if "k" in which:
                    nc.vector.tensor_copy(out=kt[b][:, nsl], in_=ps[:, 512:1024])

            def v_quad(b, sq):
                """v rows for four 128-token chunks (si = 4*sq .. 4*sq+3)."""
                ps = pspool.tile([128, 1024], fp32, tag="aux", name=f"v{b}_{sq}")
                for s4 in range(4):
                    si = 4 * sq + s4
                    msl = slice(128 * si, 128 * si + 128)
                    fsl = slice(128 * s4, 128 * s4 + 128)
                    for ci in range(8):
                        nc.tensor.matmul(
                            ps[:, fsl], lhsT=xts[b, ci][:, msl], rhs=wv[:, ci, :],
                            start=(ci == 0), stop=(ci == 7),
                        )
                for s4 in range(4):
                    si = 4 * sq + s4
                    for hi in range(HPC):
                        jsl = slice(128 * s4 + 64 * hi, 128 * s4 + 64 * hi + 64)
                        nc.vector.tensor_add(
                            out=vaug[b * HPC + hi][:, si, 0:64],
                            in0=ps[:, jsl], in1=bv[:, 64 * hi:64 * hi + 64],
                        )

            def attn_nj(b, nj, hooks=None):
                """One 512-wide query chunk: scores, exp, bias-mul, PV.
                hooks[mi] emits interleaved foreign work (A/C chunks) between
                tiles so the PE never has a head-of-line stall on one slot."""
                hooks = hooks or {}
                qsl = slice(512 * nj, 512 * nj + 512)
                pv = pspool.tile([128, 1024], fp32, tag="pv", name=f"pv{b}_{nj}")
                ebt = None
                pend = None  # deferred PV for 1-tile PE lookahead
                for mi in range(16):
                    if mi % 4 == 0:
                        ebt = ebpool.tile([128, 4, 2, 512], bfl, tag="eb", name=f"eb{b}_{nj}_{mi}")
                        nc.sync.dma_start(out=ebt[:], in_=eb_d[nj, mi // 4])
                    msl = slice(128 * mi, 128 * mi + 128)
                    sc = pspool.tile(
                        [128, 1024], fp32, tag=f"sc{mi % 2}", name=f"sc{b}_{nj}_{mi}"
                    )
                    nc.tensor.matmul(
                        sc[:, 0:512], lhsT=kt[b][0:64, msl], rhs=qt[b][0:64, qsl],
                        start=True, stop=True,
                    )
                    nc.tensor.matmul(
                        sc[:, 512:1024], lhsT=kt[b][64:128, msl],
                        rhs=qt[b][64:128, qsl], start=True, stop=True,
                    )
                    es = espool.tile([128, 1024], bfl, tag="es", name=f"es{b}_{nj}_{mi}")
                    nc.scalar.activation(out=es[:], in_=sc[:], func=EXP)
                    nc.vector.tensor_mul(
                        out=es[:], in0=es[:], in1=ebt[:, mi % 4],
                    )
                    if pend is not None:
                        _pv_pair(b, pv, *pend)
                    pend = (es, mi)
                    if mi in hooks:
                        hooks[mi]()
                _pv_pair(b, pv, *pend)
                # drain: raw attention rows + denominators (all on DVE so the
                # next nj's PV start carries a single WAR sem)
                nc.vector.tensor_copy(out=aot[b][0:64, qsl], in_=pv[0:64, 0:512])
                nc.vector.tensor_copy(out=aot[b][64:128, qsl], in_=pv[0:64, 512:1024])
                nc.vector.tensor_copy(
                    out=den_h[nj // 2][:, :, nj % 2, :], in_=pv[64:65, :]
                )

            def _pv_pair(b, pv, es, mi):
                nc.tensor.matmul(
                    pv[0:65, 0:512], lhsT=vaug[b * HPC][:, mi, :],
                    rhs=es[:, 0:512], start=(mi == 0), stop=(mi == 15),
                )
                nc.tensor.matmul(
                    pv[0:65, 512:1024], lhsT=vaug[b * HPC + 1][:, mi, :],
                    rhs=es[:, 512:1024], start=(mi == 0), stop=(mi == 15),
                )

            def recip_half(b, h2):
                """reciprocal+normalize for query columns [1024*h2, 1024*h2+1024)
                (den of njs {2*h2, 2*h2+1}).  den -> [128, 16] via SBUF->SBUF
                DMA, batched 1/x, DRAM bounce for the partition-broadcast."""
                nc.sync.dma_start(out=den_t[:], in_=den_h[h2][:])
                nc.vector.reciprocal(out=rec_t[:], in_=den_t[:])
                rec_st = bass.AP(
                    tensor=rec_d.tensor, offset=rec_d.offset + 2048 * h2,
                    ap=[[16, 128], [1, 16]],
                )
                nc.sync.dma_start(out=rec_st, in_=rec_t[:])
                csl = slice(1024 * h2, 1024 * h2 + 1024)
                for hi in range(HPC):
                    src = bass.AP(
                        tensor=rec_d.tensor,
                        offset=rec_d.offset + 2048 * h2 + 1024 * hi,
                        ap=[[0, 64], [1, 1024]],
                    )
                    nc.sync.dma_start(out=rec2[b][64 * hi:64 * hi + 64, csl], in_=src)
                nc.vector.tensor_mul(
                    out=aot[b][:, csl], in0=aot[b][:, csl], in1=rec2[b][:, csl]
                )

            def proj_si(b, si, eng, tag="aux"):
                """output projection for one 128-token chunk."""
                msl = slice(128 * si, 128 * si + 128)
                ps = pspool.tile([128, 1024], fp32, tag=tag, name=f"pj{b}_{si}")
                for half in range(2):
                    fsl = slice(512 * half, 512 * half + 512)
                    nc.tensor.matmul(
                        ps[:, fsl], lhsT=aot[b][:, msl], rhs=pw[:, fsl],
                        start=True, stop=True,
                    )
                ob = opool.tile([128, 1024], bfl, tag="ob", name=f"ob{b}_{si}")
                if eng == "act":
                    nc.scalar.copy(out=ob[:], in_=ps[:])
                elif eng == "both":
                    nc.scalar.copy(out=ob[:, 0:512], in_=ps[:, 0:512])
                    nc.vector.tensor_copy(out=ob[:, 512:1024], in_=ps[:, 512:1024])
                else:
                    nc.vector.tensor_copy(out=ob[:], in_=ps[:])
                nc.sync.dma_start(out=out_d[b, si], in_=ob[:])

            # ================= emission =================
            # A(b0): qk on rotating psum slots (PE-dense), first v quad
            load_x(0)
            for ni, tg in zip(range(4), ("aux", "sc0", "sc1", "aux")):
                qk_chunk(0, ni, tag=tg)
            v_quad(0, 0)

            # B(b0): v(b0) quads stream into nj0 just-in-time; A(b1)
            # (k fully, q chunk 0 only -- q chunks 1-3 defer into B(b1))
            # spreads over njs 1-2; x(b1) prefetch kicks off mid-nj0
            attn_nj(0, 0, hooks={
                2: lambda: v_quad(0, 1),
                6: lambda: v_quad(0, 2),
                10: lambda: (load_x(1), v_quad(0, 3)),
            })
            attn_nj(0, 1, hooks={
                3: lambda: qk_chunk(1, 0, which="k"),
                7: lambda: qk_chunk(1, 1, which="k"),
                11: lambda: v_quad(1, 0),
                15: lambda: qk_chunk(1, 2, which="k"),
            })
            recip_half(0, 0)
            attn_nj(0, 2, hooks={
                3: lambda: qk_chunk(1, 3, which="k"),
                7: lambda: v_quad(1, 1),
                11: lambda: v_quad(1, 2),
                15: lambda: v_quad(1, 3),
            })
            attn_nj(0, 3, hooks={
                8: lambda: qk_chunk(1, 0, which="q"),
            })
            recip_half(0, 1)

            # B(b1): C(b0) proj + deferred q(b1) chunks inside the mi loop
            for nj in range(4):
                hooks = {
                    2: lambda nj=nj: proj_si(0, 4 * nj + 0, "vec"),
                    6: lambda nj=nj: proj_si(0, 4 * nj + 1, "vec"),
                    10: lambda nj=nj: proj_si(0, 4 * nj + 2, "vec"),
                    14: lambda nj=nj: proj_si(0, 4 * nj + 3, "vec"),
                }
                if nj < 3:
                    hooks[12] = lambda nj=nj: qk_chunk(1, nj + 1, which="q")
                attn_nj(1, nj, hooks=hooks)
                if nj == 1:
                    recip_half(1, 0)
            recip_half(1, 1)

            # C(b1): pipelined over three psum slots, dual-engine evictions
            tags = ("sc0", "sc1", "aux")
            for si in range(16):
                proj_si(1, si, "both", tag=tags[si % 3])

    _fix_sync_waits(nc)
    return nc


def _prep_inputs(x, rel_pos_bias, qkv_w, q_bias, v_bias):
    """Build the 8 per-core input maps (host-side shard + transpose + cast)."""
    x = np.asarray(x, dtype=np.float32)
    rel_pos_bias = np.asarray(rel_pos_bias, dtype=np.float32)
    qkv_w = np.asarray(qkv_w, dtype=np.float32)
    q_bias = np.asarray(q_bias, dtype=np.float32)
    v_bias = np.asarray(v_bias, dtype=np.float32)

    # xT: [b, c, n] -> [b, 8, 128, n]
    xt = np.ascontiguousarray(x.transpose(0, 2, 1)).reshape(B, 8, 128, N).astype(bf16)

    in_maps = []
    for c in range(NCORES):
        heads = [HPC * c + i for i in range(HPC)]
        jrows = np.concatenate([np.arange(64 * h, 64 * h + 64) for h in heads])

        def tile_w(rows, scale=1.0):
            wt = (scale * qkv_w[rows]).T.astype(bf16)  # [1024 c, 128 j]
            return np.ascontiguousarray(wt.reshape(8, 128, 128).transpose(1, 0, 2))

        wq = tile_w(jrows, SCALE)
        wk = tile_w(C + jrows)
        wv = tile_w(2 * C + jrows)
        bq = (SCALE * q_bias[jrows]).reshape(128, 1).astype(np.float32)
        bv = np.ascontiguousarray(
            np.broadcast_to(v_bias[jrows][None, :], (128, 128)).astype(np.float32)
        )
        # exp of transposed bias: [h, m, n] -> [nj, miP, m^, mi4, h, n]
        ebt = np.exp(rel_pos_bias[heads].transpose(0, 2, 1))  # [hpc, m, n]
        ebt = np.ascontiguousarray(
            ebt.reshape(HPC, 4, 4, 128, 4, 512).transpose(4, 1, 3, 2, 0, 5)
        ).astype(bf16)
        in_maps.append(
            {"xt": xt, "wq": wq, "wk": wk, "wv": wv, "bq": bq, "bv": bv, "expb": ebt}
        )
    return in_maps


def kernel(x, rel_pos_bias, qkv_w, q_bias, v_bias, proj_w, proj_b):
    from concourse.bass_utils import run_bass_kernel_spmd

    x = np.asarray(x, dtype=np.float32)
    proj_w = np.asarray(proj_w, dtype=np.float32)
    proj_b = np.asarray(proj_b, dtype=np.float32)

    if "nc" not in _graph_cache:
        _graph_cache["nc"] = _build_graph()
    nc = _graph_cache["nc"]

    in_maps = _prep_inputs(x, rel_pos_bias, qkv_w, q_bias, v_bias)
    for c in range(NCORES):
        heads = [HPC * c + i for i in range(HPC)]
        jrows = np.concatenate([np.arange(64 * h, 64 * h + 64) for h in heads])
        pw = np.ascontiguousarray(proj_w[:, jrows].T.astype(bf16))  # [128 j, 1024 e]
        in_maps[c]["pw"] = pw

    res = run_bass_kernel_spmd(nc, in_maps, core_ids=list(range(NCORES)))
    out = np.zeros((B, 16, 128, C), dtype=np.float32)
    for r in res.results:
        out += np.asarray(r["out"], dtype=np.float32)
    out = out.reshape(B, N, C) + proj_b[None, None, :]
    return out


# revision 22
# speedup vs baseline: 1.1489x; 1.1489x over previous
"""Distributed attention kernel for 8 trn2 NeuronCores.

Problem: B=2, N=2048, C=1024, H=16, D=64 attention with relative position
bias, qkv projection and output projection.

Sharding: head-parallel, 2 heads per core, both batches on every core.
Each core computes a partial output projection (its 2 heads' contribution
to all 1024 output channels); the host sums the 8 partials.

v2 structure (software-pipelined, batch-sequential):
  A(b0) -> B(b0) [A(b1) chunks interleaved] -> B(b1) [C(b0) chunks
  interleaved] -> C(b1)
  - stage B processes 512-wide query chunks (nj); per (nj, mi) one psum
    tile holds both heads' scores^T ([128 m, A|B 512 n each]); ACT exp
    evicts to bf16 es; DVE multiplies exp(bias) in; two PV matmuls
    accumulate into a [65, 1024] psum pair (ones column -> denominators).
  - scores matmuls are row-packed (head A K-rows 0:64, head B 64:128)
    and run concurrently on the PE.
  - reciprocal of the 4096 per-batch denominators is batched: DRAM
    round-trip reshapes [1, 4096] -> [128, 32] so the DVE divide uses
    all partitions (the [1, N] form costs ~6.5us per call).
  - exp(bias) tiles stream from HBM as 1MB chunks (4 mi per load).
"""

import sys

import numpy as np
import ml_dtypes

sys.path.insert(0, "/opt/trn_rl_repo")

B, N, C = 2, 2048, 1024
H, D = 16, 64
SCALE = D**-0.5
NCORES = 8
HPC = H // NCORES  # heads per core = 2

bf16 = ml_dtypes.bfloat16

_graph_cache = {}


def _fix_sync_waits(nc):
    """Walrus in this toolchain accepts at most ONE sync wait on compute
    instructions (two on DMAs).  Tile emits more.  Fix up the built graph:
      - drop waits on the instruction's own scheduled proc (in-order
        execution of a proc makes them always-satisfied),
      - hoist remaining excess waits onto standalone NoOps inserted just
        before the instruction on the same engine.
    """
    from concourse import mybir
    from concourse.tile_sem_assignment import PROC_NAME_TO_IDX

    idx_to_proc = {v: k for k, v in PROC_NAME_TO_IDX.items()}
    fixid = [0]
    for fn in nc.m.functions:
        for bb in fn.blocks:
            insts = list(bb.instructions)
            out = []
            changed = False
            for inst in insts:
                si = inst.sync_info
                if not si or not si.on_wait:
                    out.append(inst)
                    continue
                own = idx_to_proc.get(inst.bass_scheduled_proc, None)
                waits = list(si.on_wait)
                if own is not None:
                    kept = [
                        w
                        for w in waits
                        if w.ant_name.rsplit("_", 1)[0] != own
                    ]
                else:
                    kept = waits
                limit = 1
                hoist = []
                if len(kept) > limit:
                    hoist = kept[: len(kept) - limit]
                    kept = kept[len(kept) - limit :]
                if len(kept) != len(waits) or hoist:
                    changed = True
                    for w in hoist:
                        fixid[0] += 1
                        nop = mybir.InstNoOp(
                            name=f"W-fix-{fixid[0]}",
                            ins=[],
                            outs=[],
                            engine=inst.engine,
                            bass_nofuse=True,
                            text_hint="wait-split",
                            sync_info=mybir.SyncInfo(on_wait=[w], on_update=[]),
                        )
                        out.append(nop)
                    si.on_wait = kept
                out.append(inst)
            if changed:
                bb.instructions = out


def _build_graph():
    import concourse.bass as bass
    import concourse.tile as tile
    from concourse import mybir

    EXP = mybir.ActivationFunctionType.Exp
    fp32 = mybir.dt.float32
    bfl = mybir.dt.bfloat16

    nc = bass.Bass()

    xt_d = nc.declare_dram_parameter("xt", [B, 8, 128, N], bfl, isOutput=False)
    wq_d = nc.declare_dram_parameter("wq", [128, 8, 128], bfl, isOutput=False)
    wk_d = nc.declare_dram_parameter("wk", [128, 8, 128], bfl, isOutput=False)
    wv_d = nc.declare_dram_parameter("wv", [128, 8, 128], bfl, isOutput=False)
    bq_d = nc.declare_dram_parameter("bq", [128, 1], fp32, isOutput=False)
    bv_d = nc.declare_dram_parameter("bv", [128, 1], fp32, isOutput=False)
    # exp(bias^T): [nj 4][miP 4][m^ 128][mi4 4][h 2][n 512]
    eb_d = nc.declare_dram_parameter(
        "expb", [4, 4, 128, 4, 2, 512], bfl, isOutput=False
    )
    pw_d = nc.declare_dram_parameter("pw", [128, 1024], bfl, isOutput=False)
    out_d = nc.declare_dram_parameter("out", [B, 16, 128, C], bfl, isOutput=True)

    with tile.TileContext(nc) as tc:
        with (
            tc.tile_pool(name="weights", bufs=1) as wpool,
            tc.tile_pool(name="xt", bufs=1) as xpool,
            tc.tile_pool(name="qkvt", bufs=1) as qkpool,
            tc.tile_pool(name="eb", bufs=4) as ebpool,
            tc.tile_pool(name="es", bufs=10) as espool,
            tc.tile_pool(name="outsb", bufs=6) as opool,
            tc.tile_pool(name="ps", bufs=1, space="PSUM") as pspool,
            tc.tile_pool(name="dramsc", bufs=1, space="DRAM") as dpool,
        ):
            # ---- persistent weights ----
            wq = wpool.tile([128, 8, 128], bfl)
            wk = wpool.tile([128, 8, 128], bfl)
            wv = wpool.tile([128, 8, 128], bfl)
            pw = wpool.tile([128, 1024], bfl)
            bq = wpool.tile([128, 1], fp32)
            bvc = wpool.tile([128, 1], fp32)
            nc.sync.dma_start(out=wq[:], in_=wq_d[:])
            nc.sync.dma_start(out=wk[:], in_=wk_d[:])
            nc.sync.dma_start(out=wv[:], in_=wv_d[:])
            nc.sync.dma_start(out=pw[:], in_=pw_d[:])
            nc.sync.dma_start(out=bq[:], in_=bq_d[:])
            nc.sync.dma_start(out=bvc[:], in_=bv_d[:])

            qt = [qkpool.tile([128, N], bfl, tag=f"qt{b}", name=f"qt{b}") for b in range(B)]
            kt = [qkpool.tile([128, N], bfl, tag=f"kt{b}", name=f"kt{b}") for b in range(B)]
            aot = [qkpool.tile([128, N], bfl, tag=f"aot{b}", name=f"aot{b}") for b in range(B)]
            rec2 = [qkpool.tile([128, N], fp32, tag=f"rec2{b}", name=f"rec2{b}") for b in range(B)]
            # padded to 128 so the xbar transpose lands on aligned strides;
            # PV lhsT slices [:, mi, 0:65] (64 v cols + ones column)
            vaug = [
                qkpool.tile([128, 16, 128], bfl, tag=f"vaug{u}", name=f"vaug{u}") for u in range(B * HPC)
            ]
            for u in range(B * HPC):
                nc.vector.memset(vaug[u][:, :, 64:65], 1.0)
            # denominators, one tile per 1024-wide query half: [h, nj', n]
            den_h = [
                qkpool.tile([1, 2, 2, 512], fp32, tag=f"den{i}", name=f"den{i}")
                for i in range(2)
            ]
            vts = [qkpool.tile([128, N], bfl, tag=f"vts{b}", name=f"vts{b}") for b in range(B)]
            den_t = qkpool.tile([128, 16], fp32, tag="den_t", name="den_t")
            rec_t = qkpool.tile([128, 16], fp32, tag="rec_t", name="rec_t")

            rec_d = dpool.tile([1, 4096], fp32, tag="rec_d", name="rec_d")

            warm = wpool.tile([128, 128], bfl)
            nc.vector.memset(warm[:], 1.0)

            def pe_warmup(n):
                ps = pspool.tile([128, 1024], fp32, tag="aux", name="warmps")
                for _ in range(n):
                    nc.tensor.matmul(
                        ps[:, 0:128], lhsT=warm[:], rhs=warm[:],
                        start=True, stop=True,
                    )

            xts = {}

            def load_x(b, njc=None):
                """Load x tiles; njc selects one 512-wide column chunk so the
                first qk/vt chunks can start before the full x arrives."""
                for ci in range(8):
                    if njc is None or njc == 0:
                        t = xpool.tile([128, N], bfl, tag=f"xt{ci}", name=f"x{b}_{ci}")
                        xts[b, ci] = t
                    t = xts[b, ci]
                    if njc is None:
                        nc.sync.dma_start(out=t[:], in_=xt_d[b, ci])
                    else:
                        nsl = slice(512 * njc, 512 * njc + 512)
                        nc.sync.dma_start(out=t[:, nsl], in_=xt_d[b, ci][:, nsl])

            def qk_chunk(b, ni, tag="aux", which="qk"):
                """q^T and/or k^T for one 512-wide token chunk."""
                nsl = slice(512 * ni, 512 * ni + 512)
                ps = pspool.tile([128, 1024], fp32, tag=tag, name=f"qk{b}_{ni}")
                for ci in range(8):
                    if "q" in which:
                        nc.tensor.matmul(
                            ps[:, 0:512], lhsT=wq[:, ci, :], rhs=xts[b, ci][:, nsl],
                            start=(ci == 0), stop=(ci == 7),
                        )
                    if "k" in which:
                        nc.tensor.matmul(
                            ps[:, 512:1024], lhsT=wk[:, ci, :], rhs=xts[b, ci][:, nsl],
                            start=(ci == 0), stop=(ci == 7),
                        )
                if "q" in which:
                    nc.vector.tensor_scalar_add(
                        out=qt[b][:, nsl], in0=ps[:, 0:512], scalar1=bq[:]
                    )
                if "k" in which:
                    nc.vector.tensor_copy(out=kt[b][:, nsl], in_=ps[:, 512:1024])

            def vt_half(b, half, tag="aux"):
                """v^T for two 512-token quads, then DMA-transpose into the
                natural-layout vaug tiles (16 [64,128]->[128,64] xbar moves)."""
                ps = pspool.tile([128, 1024], fp32, tag=tag, name=f"vt{b}_{half}")
                for q2 in range(2):
                    quad = 2 * half + q2
                    msl = slice(512 * quad, 512 * quad + 512)
                    fsl = slice(512 * q2, 512 * q2 + 512)
                    for ci in range(8):
                        nc.tensor.matmul(
                            ps[:, fsl], lhsT=wv[:, ci, :], rhs=xts[b, ci][:, msl],
                            start=(ci == 0), stop=(ci == 7),
                        )
                for q2 in range(2):
                    quad = 2 * half + q2
                    nc.vector.tensor_scalar_add(
                        out=vts[b][:, 512 * quad:512 * quad + 512],
                        in0=ps[:, 512 * q2:512 * q2 + 512], scalar1=bvc[:],
                    )
                hsl = slice(1024 * half, 1024 * half + 1024)
                for hi in range(HPC):
                    nc.sync.dma_start_transpose(
                        out=vaug[b * HPC + hi][:, 8 * half:8 * half + 8, 0:64],
                        in_=vts[b][64 * hi:64 * hi + 64, hsl],
                    )

            def attn_nj(b, nj, hooks=None):
                """One 512-wide query chunk: scores, exp, bias-mul, PV.
                hooks[mi] emits interleaved foreign work (A/C chunks) between
                tiles so the PE never has a head-of-line stall on one slot."""
                hooks = hooks or {}
                qsl = slice(512 * nj, 512 * nj + 512)
                pv = pspool.tile([128, 1024], fp32, tag="pv", name=f"pv{b}_{nj}")
                ebt = None
                pend = None  # deferred PV for 1-tile PE lookahead
                for mi in range(16):
                    if mi % 4 == 0:
                        ebt = ebpool.tile([128, 4, 2, 512], bfl, tag="eb", name=f"eb{b}_{nj}_{mi}")
                        nc.sync.dma_start(out=ebt[:], in_=eb_d[nj, mi // 4])
                    msl = slice(128 * mi, 128 * mi + 128)
                    sc = pspool.tile(
                        [128, 1024], fp32, tag=f"sc{mi % 2}", name=f"sc{b}_{nj}_{mi}"
                    )
                    nc.tensor.matmul(
                        sc[:, 0:512], lhsT=kt[b][0:64, msl], rhs=qt[b][0:64, qsl],
                        start=True, stop=True,
                    )
                    nc.tensor.matmul(
                        sc[:, 512:1024], lhsT=kt[b][64:128, msl],
                        rhs=qt[b][64:128, qsl], start=True, stop=True,
                    )
                    es = espool.tile([128, 1024], bfl, tag="es", name=f"es{b}_{nj}_{mi}")
                    nc.scalar.activation(out=es[:], in_=sc[:], func=EXP)
                    nc.vector.tensor_mul(
                        out=es[:], in0=es[:], in1=ebt[:, mi % 4],
                    )
                    if pend is not None:
                        _pv_pair(b, pv, *pend)
                    pend = (es, mi)
                    if mi in hooks:
                        hooks[mi]()
                _pv_pair(b, pv, *pend)
                # drain: raw attention rows + denominators (all on DVE so the
                # next nj's PV start carries a single WAR sem)
                nc.vector.tensor_copy(out=aot[b][0:64, qsl], in_=pv[0:64, 0:512])
                nc.vector.tensor_copy(out=aot[b][64:128, qsl], in_=pv[0:64, 512:1024])
                nc.vector.tensor_copy(
                    out=den_h[nj // 2][:, :, nj % 2, :], in_=pv[64:65, :]
                )

            def _pv_pair(b, pv, es, mi):
                nc.tensor.matmul(
                    pv[0:65, 0:512], lhsT=vaug[b * HPC][:, mi, 0:65],
                    rhs=es[:, 0:512], start=(mi == 0), stop=(mi == 15),
                )
                nc.tensor.matmul(
                    pv[0:65, 512:1024], lhsT=vaug[b * HPC + 1][:, mi, 0:65],
                    rhs=es[:, 512:1024], start=(mi == 0), stop=(mi == 15),
                )

            def recip_half(b, h2):
                """reciprocal+normalize for query columns [1024*h2, 1024*h2+1024)
                (den of njs {2*h2, 2*h2+1}).  den -> [128, 16] via SBUF->SBUF
                DMA, batched 1/x, DRAM bounce for the partition-broadcast."""
                nc.sync.dma_start(out=den_t[:], in_=den_h[h2][:])
                nc.vector.reciprocal(out=rec_t[:], in_=den_t[:])
                rec_st = bass.AP(
                    tensor=rec_d.tensor, offset=rec_d.offset + 2048 * h2,
                    ap=[[16, 128], [1, 16]],
                )
                nc.sync.dma_start(out=rec_st, in_=rec_t[:])
                csl = slice(1024 * h2, 1024 * h2 + 1024)
                for hi in range(HPC):
                    src = bass.AP(
                        tensor=rec_d.tensor,
                        offset=rec_d.offset + 2048 * h2 + 1024 * hi,
                        ap=[[0, 64], [1, 1024]],
                    )
                    nc.sync.dma_start(out=rec2[b][64 * hi:64 * hi + 64, csl], in_=src)
                nc.vector.tensor_mul(
                    out=aot[b][:, csl], in0=aot[b][:, csl], in1=rec2[b][:, csl]
                )

            def proj_si(b, si, eng, tag="aux"):
                """output projection for one 128-token chunk."""
                msl = slice(128 * si, 128 * si + 128)
                ps = pspool.tile([128, 1024], fp32, tag=tag, name=f"pj{b}_{si}")
                for half in range(2):
                    fsl = slice(512 * half, 512 * half + 512)
                    nc.tensor.matmul(
                        ps[:, fsl], lhsT=aot[b][:, msl], rhs=pw[:, fsl],
                        start=True, stop=True,
                    )
                ob = opool.tile([128, 1024], bfl, tag="ob", name=f"ob{b}_{si}")
                if eng == "act":
                    nc.scalar.copy(out=ob[:], in_=ps[:])
                elif eng == "both":
                    nc.scalar.copy(out=ob[:, 0:512], in_=ps[:, 0:512])
                    nc.vector.tensor_copy(out=ob[:, 512:1024], in_=ps[:, 512:1024])
                else:
                    nc.vector.tensor_copy(out=ob[:], in_=ps[:])
                nc.sync.dma_start(out=out_d[b, si], in_=ob[:])

            # ================= emission =================
            # A(b0) minimal prefix: only what B(b0) nj0 mi0-7 needs (q/k
            # chunks 0-1 cover keys mi 0-7; both v^T halves for PV); the
            # rest streams in as nj0 hooks.
            load_x(0, njc=0)
            pe_warmup(48)
            qk_chunk(0, 0, tag="aux")
            load_x(0, njc=1)
            qk_chunk(0, 1, tag="sc0")
            vt_half(0, 0, tag="sc1")
            load_x(0, njc=2)
            load_x(0, njc=3)
            vt_half(0, 1, tag="aux")

            # B(b0): A(b1) spreads over njs 0-2 (k fully, v^T, q chunk 0;
            # q chunks 1-3 defer into B(b1)); x(b1) prefetch at nj0 end
            attn_nj(0, 0, hooks={
                3: lambda: qk_chunk(0, 2),
                7: lambda: qk_chunk(0, 3),
                15: lambda: load_x(1),
            })
            attn_nj(0, 1, hooks={
                3: lambda: qk_chunk(1, 0, which="k"),
                7: lambda: qk_chunk(1, 1, which="k"),
                11: lambda: qk_chunk(1, 2, which="k"),
                15: lambda: qk_chunk(1, 3, which="k"),
            })
            recip_half(0, 0)
            attn_nj(0, 2, hooks={
                3: lambda: vt_half(1, 0),
                8: lambda: vt_half(1, 1),
                13: lambda: qk_chunk(1, 0, which="q"),
            })
            attn_nj(0, 3)
            recip_half(0, 1)

            # B(b1): C(b0) proj + deferred q(b1) chunks inside the mi loop
            for nj in range(4):
                hooks = {
                    2: lambda nj=nj: proj_si(0, 4 * nj + 0, "vec"),
                    6: lambda nj=nj: proj_si(0, 4 * nj + 1, "vec"),
                    10: lambda nj=nj: proj_si(0, 4 * nj + 2, "vec"),
                    14: lambda nj=nj: proj_si(0, 4 * nj + 3, "vec"),
                }
                if nj < 3:
                    hooks[12] = lambda nj=nj: qk_chunk(1, nj + 1, which="q")
                else:
                    hooks[4] = lambda: proj_si(1, 0, "both")
                    hooks[8] = lambda: proj_si(1, 1, "both")
                    hooks[12] = lambda: proj_si(1, 2, "both")
                attn_nj(1, nj, hooks=hooks)
                if nj == 1:
                    recip_half(1, 0)
            recip_half(1, 1)

            # C(b1): pipelined over three psum slots, dual-engine evictions
            tags = ("sc0", "sc1", "aux")
            for si in range(3, 16):
                proj_si(1, si, "both", tag=tags[si % 3])

    _fix_sync_waits(nc)
    return nc


def _prep_inputs(x, rel_pos_bias, qkv_w, q_bias, v_bias):
    """Build the 8 per-core input maps (host-side shard + transpose + cast)."""
    x = np.asarray(x, dtype=np.float32)
    rel_pos_bias = np.asarray(rel_pos_bias, dtype=np.float32)
    qkv_w = np.asarray(qkv_w, dtype=np.float32)
    q_bias = np.asarray(q_bias, dtype=np.float32)
    v_bias = np.asarray(v_bias, dtype=np.float32)

    # xT: [b, c, n] -> [b, 8, 128, n]
    xt = np.ascontiguousarray(x.transpose(0, 2, 1)).reshape(B, 8, 128, N).astype(bf16)

    in_maps = []
    for c in range(NCORES):
        heads = [HPC * c + i for i in range(HPC)]
        jrows = np.concatenate([np.arange(64 * h, 64 * h + 64) for h in heads])

        def tile_w(rows, scale=1.0):
            wt = (scale * qkv_w[rows]).T.astype(bf16)  # [1024 c, 128 j]
            return np.ascontiguousarray(wt.reshape(8, 128, 128).transpose(1, 0, 2))

        wq = tile_w(jrows, SCALE)
        wk = tile_w(C + jrows)
        wv = tile_w(2 * C + jrows)
        bq = (SCALE * q_bias[jrows]).reshape(128, 1).astype(np.float32)
        bv = v_bias[jrows].reshape(128, 1).astype(np.float32)
        # exp of transposed bias: [h, m, n] -> [nj, miP, m^, mi4, h, n]
        ebt = np.exp(rel_pos_bias[heads].transpose(0, 2, 1))  # [hpc, m, n]
        ebt = np.ascontiguousarray(
            ebt.reshape(HPC, 4, 4, 128, 4, 512).transpose(4, 1, 3, 2, 0, 5)
        ).astype(bf16)
        in_maps.append(
            {"xt": xt, "wq": wq, "wk": wk, "wv": wv, "bq": bq, "bv": bv, "expb": ebt}
        )
    return in_maps


def kernel(x, rel_pos_bias, qkv_w, q_bias, v_bias, proj_w, proj_b):
    from concourse.bass_utils import run_bass_kernel_spmd

    x = np.asarray(x, dtype=np.float32)
    proj_w = np.asarray(proj_w, dtype=np.float32)
    proj_b = np.asarray(proj_b, dtype=np.float32)

    if "nc" not in _graph_cache:
        _graph_cache["nc"] = _build_graph()
    nc = _graph_cache["nc"]

    in_maps = _prep_inputs(x, rel_pos_bias, qkv_w, q_bias, v_bias)
    for c in range(NCORES):
        heads = [HPC * c + i for i in range(HPC)]
        jrows = np.concatenate([np.arange(64 * h, 64 * h + 64) for h in heads])
        pw = np.ascontiguousarray(proj_w[:, jrows].T.astype(bf16))  # [128 j, 1024 e]
        in_maps[c]["pw"] = pw

    res = run_bass_kernel_spmd(nc, in_maps, core_ids=list(range(NCORES)))
    out = np.zeros((B, 16, 128, C), dtype=np.float32)
    for r in res.results:
        out += np.asarray(r["out"], dtype=np.float32)
    out = out.reshape(B, N, C) + proj_b[None, None, :]
    return out
